# revision 26
# baseline (speedup 1.0000x reference)
"""Trainium2 Bass kernel for fused attention prefill (nn_Attn_50740743635107).

Reference computation (fp32):
  qkv = x @ W_qkv.T ; split q,k,v ; interleaved RoPE on q,k ;
  scores = q k^T / sqrt(dh) with causal+valid_k mask ; softmax ;
  ctx = attn @ v ; out = ctx @ W_out.T

Shapes: B=4, S=1024, D=2048, H=16, DH=128.  seq_lens <= 928 for 3 of the
4 batches, 923 for batch 2 (deterministic inputs), but the kernel only
assumes seq_len >= 512 (mask data handles the rest).

Sharding: 8 cores = 4 batches x 2 head-groups (8 heads each).
Each core computes a partial out^T [D, S] for its (batch, head-group);
the host sums the two head-group partials per batch and transposes.

v2 design notes (vs the f32r v1):
- Everything bf16 on the PE/DVE paths: weights, x, q/k/v, exp(scores),
  ctx, W_out, output partials.  PSUM stays f32.  Halves LDWEIGHTS time,
  DVE time and DMA bytes; matmul row rate is 1 cycle/row either way.
- Host passes x^T and W^T layouts so every matmul contraction dim lands
  on SBUF partitions; q/k rows of W_qkv host-permuted so RoPE interleave
  becomes contiguous halves; 1/sqrt(dh) folded into W_q.
- k-projection only computes tokens < 928 (tail memset to 0; bias masks
  it), q full.
- exp: ONE activation per score tile (bias = valid_k column mask);
  causal masking by multiplying the diagonal 128x128 block of exp by a
  0/1 lower-triangular bf16 tile on DVE (scores are small, exp can't
  overflow, so masking after exp is exact).
- softmax denominator via ones-stationary matmul (broadcast across
  partitions); 1/den via DVE reciprocal_approx_fast (5x faster than the
  exact one, ~18 bits).
- PE warm-up: a run of dummy matmuls on the first-arrived const tile
  ramps the tensor-engine p-state while input DMAs land.
- Attention runs sh-major; the sq<512 half of the output projection is
  interleaved into the sq>=512 attention stream so the PE never waits
  on the ACT exp chain.
"""

import numpy as np
import ml_dtypes

import concourse.bass as bass
from concourse import bacc
import concourse.mybir as mybir
import concourse.tile as tile
from concourse.bass_utils import run_bass_kernel_spmd

B, S, D, H = 4, 1024, 2048, 16
DH = 128           # head dim
HPC = 8            # heads per core
DC = HPC * DH      # 1024: d-range per core
P = 128
KTOK = 928         # k tokens computed (>= max seq_len, multiple of 16)
THETA = 10000.0
NEG = -60.0        # additive mask for invalid-k positions
F32 = mybir.dt.float32
BF16 = mybir.dt.bfloat16
MULT = mybir.AluOpType.mult
ADD = mybir.AluOpType.add
SUB = mybir.AluOpType.subtract
EXP = mybir.ActivationFunctionType.Exp
COPY = mybir.ActivationFunctionType.Copy

# score tiles per head: (sk_tile, sq_half) pairs that are (partially) allowed.
# sq_half h covers sq in [512h, 512h+512); sk tile t covers sk in [128t, ...).
# diagonal (need causal mask): sh=0: t=0..3 ; sh=1: t=4..7.
# full (no mask): sh=1: t=0..3.
DIAG = {(t, 0) for t in range(4)} | {(t, 1) for t in range(4, 8)}
ALLOWED = {0: [0, 1, 2, 3], 1: [0, 1, 2, 3, 4, 5, 6, 7]}  # sq_half -> sk tiles


def build_nc(ktok=KTOK):
    nc = bacc.Bacc()
    xT_d = nc.dram_tensor("xT", [D, S], BF16, kind="ExternalInput")
    # w1m[p, mt, kt, m] = W^T[kt*128+p, mt*128+m]: every [P,16,128] weight
    # tile is contiguous per partition (fast DMA, one descriptor/partition)
    w1m_d = nc.dram_tensor("w1m", [P, 24, 16, P], BF16, kind="ExternalInput")
    wom_d = nc.dram_tensor("wom", [P, 8, D], BF16, kind="ExternalInput")
    cs_d = nc.dram_tensor("cs", [P, 2, S], BF16, kind="ExternalInput")
    tri_d = nc.dram_tensor("tri", [P, P], BF16, kind="ExternalInput")
    bias_d = nc.dram_tensor("bias", [P, 8], F32, kind="ExternalInput")
    ones_d = nc.dram_tensor("ones", [P, P], BF16, kind="ExternalInput")
    outT_d = nc.dram_tensor("outT", [D, S], BF16, kind="ExternalOutput")

    with tile.TileContext(nc) as tc:
        with (
            tc.tile_pool(name="qkt", bufs=1) as qktp,      # [128,16,1024] bf16 32K/p
            tc.tile_pool(name="vsb", bufs=1) as vsbp,      # [128,8,1024] bf16 16K/p
            tc.tile_pool(name="cstb", bufs=1) as cstbp,    # tri/bias/ones consts
        ):
            qkT = qktp.tile([P, 16, S], BF16, tag="qkt")
            vsb = vsbp.tile([P, 8, DC], BF16, tag="vsb")
            # consts on the gpsimd queue (cheap issue) so the scalar queue
            # starts streaming wqk0 immediately
            tri_t = cstbp.tile([P, P], BF16, tag="tri")
            nc.gpsimd.dma_start(tri_t[:], tri_d[:])
            bias_t = cstbp.tile([P, 8], F32, tag="bias")
            nc.gpsimd.dma_start(bias_t[:], bias_d[:])
            ones_t = cstbp.tile([P, P], BF16, tag="ones")
            nc.gpsimd.dma_start(ones_t[:], ones_d[:])

            # ---- PE p-state warm-up: dummy matmuls on a memset tile (no DMA
            # dependency) while the input DMAs land. Output is never read.
            with (
                tc.tile_pool(name="wsb", bufs=1) as wsbp,
                tc.tile_pool(name="warm", bufs=2, space=bass.MemorySpace.PSUM) as wps,
            ):
                warm_sb = wsbp.tile([P, 512], BF16, tag="wsb")
                nc.vector.memset(warm_sb[:], 0.5)
                for w in range(20):
                    wp = wps.tile([P, 512], F32, tag="warm")
                    nc.tensor.matmul(
                        wp[:], warm_sb[:, 0:P], warm_sb[:], start=True, stop=True
                    )

            # ================= phase A: QKV projection + RoPE =================
            with (
                tc.tile_pool(name="xt", bufs=1) as xtp,      # [128,16,1024] bf16 32K/p
                tc.tile_pool(name="wqk", bufs=3) as wqkp,    # [128,16,128] bf16 4K/p
                tc.tile_pool(name="wv", bufs=2) as wvp,      # [128,16,512] bf16 16K/p
                tc.tile_pool(name="cst", bufs=1) as cstp,    # cos/sin 4K/p
                tc.tile_pool(name="rope", bufs=2) as ropep,  # [128,1024] bf16 2K/p
                tc.tile_pool(name="psa", bufs=4, space=bass.MemorySpace.PSUM) as psap,
                tc.tile_pool(name="psv", bufs=2, space=bass.MemorySpace.PSUM) as psvp,
            ):
                xt = xtp.tile([P, 16, S], BF16, tag="xt")
                # zero the uncomputed k tail (tokens ktok..S of every k head)
                if ktok < S:
                    nc.vector.memset(qkT[:, 8:16, ktok:S], 0.0)
                wqks = []
                with tc.high_priority():
                    for pre in range(2):
                        wqk_p = wqkp.tile(
                            [P, 16, P], BF16, tag="wqk", name=f"wqk_pre{pre}"
                        )
                        nc.scalar.dma_start(wqk_p[:], w1m_d[:, pre])
                        wqks.append(wqk_p)
                    for kt in range(16):
                        eng = nc.sync if kt % 2 == 0 else nc.gpsimd
                        eng.dma_start(xt[:, kt, :], xT_d[P * kt : P * (kt + 1), :])
                cs_t = cstp.tile([P, 2, S], BF16, tag="cs")
                nc.scalar.dma_start(cs_t[:], cs_d[:])

                # ---- q/k projection (m-tile mt: 0..7 = q heads, 8..15 = k heads)
                # qkT[m, s] = sum_d w1T[d, m] * xT[d, s]
                for mt in range(16):
                    if mt < 2:
                        wqk = wqks[mt]
                    else:
                        wqk = wqkp.tile([P, 16, P], BF16, tag="wqk")
                        nc.gpsimd.dma_start(wqk[:], w1m_d[:, mt])
                    is_k = mt >= 8
                    n1 = (ktok - 512) if is_k else 512  # second-half cols
                    ps0 = psap.tile([P, 512], F32, tag="psa", name=f"qk{mt}_0")
                    ps1 = psap.tile([P, 512], F32, tag="psa", name=f"qk{mt}_1")
                    for kt in range(16):
                        nc.tensor.matmul(
                            ps0[:], wqk[:, kt, :], xt[:, kt, 0:512],
                            start=(kt == 0), stop=(kt == 15),
                        )
                        nc.tensor.matmul(
                            ps1[:, 0:n1], wqk[:, kt, :], xt[:, kt, 512 : 512 + n1],
                            start=(kt == 0), stop=(kt == 15),
                        )
                    nc.scalar.activation(qkT[:, mt, 0:512], ps0[:], COPY)
                    nc.scalar.activation(
                        qkT[:, mt, 512 : 512 + n1], ps1[:, 0:n1], COPY
                    )
                    # ---- RoPE in place on qkT[:, mt, :] once both halves done.
                    # rows 0..63 = even dh (xe), 64..127 = odd dh (xo):
                    #   new_e = xe*cos - xo*sin ; new_o = xe*sin + xo*cos
                    tmp = ropep.tile([P, S], BF16, tag="rope")
                    col = qkT[:, mt, :]
                    nc.vector.tensor_tensor(
                        tmp[0:64, :], col[64:128, :], cs_t[64:128, 1, :], op=MULT
                    )
                    nc.vector.tensor_tensor(
                        tmp[64:128, :], col[0:64, :], cs_t[0:64, 1, :], op=MULT
                    )
                    nc.vector.tensor_tensor(col[:], col[:], cs_t[:, 0, :], op=MULT)
                    nc.vector.tensor_tensor(
                        col[0:64, :], col[0:64, :], tmp[0:64, :], op=SUB
                    )
                    nc.vector.tensor_tensor(
                        col[64:128, :], col[64:128, :], tmp[64:128, :], op=ADD
                    )

                # ---- v projection: v[s, vd] = sum_d xT[d, s] * w1T[d, 2048+vd]
                # 512-wide moving chunks (full PSUM bank per matmul)
                for nh in range(2):
                    wv = wvp.tile([P, 4, 16, P], BF16, tag="wv")
                    nc.gpsimd.dma_start(wv[:], w1m_d[:, 16 + 4 * nh : 20 + 4 * nh])
                    for st in range(8):
                        psv = psvp.tile([P, 512], F32, tag="psv")
                        for kt in range(16):
                            nc.tensor.matmul(
                                psv[:],
                                xt[:, kt, P * st : P * (st + 1)],
                                wv[:, :, kt, :],
                                start=(kt == 0),
                                stop=(kt == 15),
                            )
                        nc.scalar.activation(
                            vsb[:, st, 512 * nh : 512 * (nh + 1)], psv[:], COPY
                        )

            # ============ phase B: attention + output projection ============
            with (
                tc.tile_pool(name="ctx", bufs=1) as ctxp,    # [128,8,1024] bf16 16K/p
                tc.tile_pool(name="ex", bufs=4) as exps,     # [128,512] bf16 1K/p
                tc.tile_pool(name="rc", bufs=2) as rcp,      # [128,512] f32 2K/p
                tc.tile_pool(name="wo", bufs=1) as wop,      # [128,8,2048] bf16 32K/p
                tc.tile_pool(name="ot", bufs=3) as otp,      # [128,512] bf16 1K/p
                tc.tile_pool(name="ps", bufs=3, space=bass.MemorySpace.PSUM) as psp,
                tc.tile_pool(name="psc", bufs=2, space=bass.MemorySpace.PSUM) as pscp,
                tc.tile_pool(name="psd", bufs=1, space=bass.MemorySpace.PSUM) as psdp,
                tc.tile_pool(name="pso", bufs=2, space=bass.MemorySpace.PSUM) as psop,
            ):
                ctxT = ctxp.tile([P, 8, S], BF16, tag="ctx")
                # W_out^T resident for the output projection
                wo_t = wop.tile([P, 8, D], BF16, tag="wo")
                nc.sync.dma_start(wo_t[:], wom_d[:])

                # ---- attention, software-pipelined and sh-major: scores
                # issue LOOKAHEAD items ahead of their exp/ctx/den; the sh=0
                # half of the output projection is interleaved into the sh=1
                # attention stream to keep the PE busy under the ACT chain.
                work = []  # (h, sh, t, i, ntiles)
                for sh in range(2):
                    for h in range(8):
                        tiles = ALLOWED[sh]
                        for i, t in enumerate(tiles):
                            work.append((h, sh, t, i, len(tiles)))

                LOOKAHEAD = 2
                scs = {}
                groups = {}  # (h, sh) -> (ctx_ps, den_ps)

                def issue_score(j):
                    h, sh, t, i, _n = work[j]
                    c0 = P * t - 512 * sh if (t, sh) in DIAG else 0
                    sc = psp.tile([P, 512], F32, tag="ps")
                    nc.tensor.matmul(
                        sc[:, c0:512],
                        qkT[:, 8 + h, P * t : P * (t + 1)],
                        qkT[:, h, 512 * sh + c0 : 512 * (sh + 1)],
                        start=True,
                        stop=True,
                    )
                    scs[j] = sc

                def out_proj(me, sh):
                    # outT[e, sq] = sum_d woT[d, e] * ctxT[d, sq]
                    ps = psop.tile([P, 512], F32, tag="pso")
                    for kd in range(8):
                        nc.tensor.matmul(
                            ps[:],
                            wo_t[:, kd, P * me : P * (me + 1)],
                            ctxT[:, kd, 512 * sh : 512 * (sh + 1)],
                            start=(kd == 0),
                            stop=(kd == 7),
                        )
                    ot = otp.tile([P, 512], BF16, tag="ot")
                    if me % 2 == 0:
                        nc.scalar.activation(ot[:], ps[:], COPY)
                    else:
                        nc.vector.tensor_copy(ot[:], ps[:])
                    # split the store across two queues to shorten the tail
                    d0 = outT_d[P * me : P * (me + 1), 512 * sh : 512 * sh + 256]
                    d1 = outT_d[P * me : P * (me + 1), 512 * sh + 256 : 512 * (sh + 1)]
                    if me % 2 == 0:
                        nc.sync.dma_start(d0, ot[:, 0:256])
                        nc.gpsimd.dma_start(d1, ot[:, 256:512])
                    else:
                        nc.gpsimd.dma_start(d0, ot[:, 0:256])
                        nc.sync.dma_start(d1, ot[:, 256:512])

                me_sh0 = 0  # next out-proj column tile for the sh=0 half
                for j in range(min(LOOKAHEAD, len(work))):
                    issue_score(j)
                for j, (h, sh, t, i, ntiles) in enumerate(work):
                    if j + LOOKAHEAD < len(work):
                        issue_score(j + LOOKAHEAD)
                    sc = scs.pop(j)
                    diag = (t, sh) in DIAG
                    c0 = P * t - 512 * sh if diag else 0
                    ex = exps.tile([P, 512], BF16, tag="ex")
                    bias = bias_t[:, t : t + 1] if (sh == 1 and t >= 4) else 0.0
                    nc.scalar.activation(ex[:, c0:512], sc[:, c0:512], EXP, bias=bias)
                    if diag:
                        # causal mask: zero the upper triangle of the
                        # diagonal 128x128 block (exact: 0 * finite = 0)
                        nc.vector.tensor_tensor(
                            ex[:, c0 : c0 + P], ex[:, c0 : c0 + P], tri_t[:], op=MULT
                        )
                    if i == 0:
                        ctx_ps = pscp.tile([P, 512], F32, tag="psc", name=f"ctxps_{h}_{sh}")
                        den_ps = psdp.tile([P, 512], F32, tag="psd", name=f"denps_{h}_{sh}")
                        groups[(h, sh)] = (ctx_ps, den_ps)
                    ctx_ps, den_ps = groups[(h, sh)]
                    first, last = (i == 0), (i == ntiles - 1)
                    nc.tensor.matmul(
                        ctx_ps[:, c0:512],
                        vsb[:, t, DH * h : DH * (h + 1)],
                        ex[:, c0:512],
                        start=first,
                        stop=last,
                    )
                    nc.tensor.matmul(
                        den_ps[:, c0:512], ones_t[:], ex[:, c0:512], start=first, stop=last
                    )
                    if last:
                        rc = rcp.tile([P, 512], F32, tag="rc")
                        nc.vector.reciprocal_approx_fast(rc[:], den_ps[:])
                        nc.vector.tensor_tensor(
                            ctxT[:, h, 512 * sh : 512 * (sh + 1)],
                            ctx_ps[:],
                            rc[:],
                            op=MULT,
                        )
                    # interleave the sh=0 output projection into the sh=1
                    # attention stream (2 column tiles per work item)
                    if sh == 1 and i >= ntiles - 2 and me_sh0 < 16:
                        out_proj(me_sh0, 0)
                        me_sh0 += 1

                while me_sh0 < 16:
                    out_proj(me_sh0, 0)
                    me_sh0 += 1
                for me in range(16):
                    out_proj(me, 1)
    nc.finalize()
    return nc


_NC_CACHE = {}


def get_nc(ktok=KTOK):
    if ktok not in _NC_CACHE:
        _NC_CACHE[ktok] = build_nc(ktok)
    return _NC_CACHE[ktok]


def make_in_maps(in_features, attention_mask, W_qkv, W_out):
    BF = ml_dtypes.bfloat16
    x = np.asarray(in_features, np.float32)
    am = np.asarray(attention_mask)
    Wqkv = np.asarray(W_qkv, np.float32)
    Wout = np.asarray(W_out, np.float32)
    seq_lens = am.astype(np.int64).sum(-1)

    perm = np.concatenate([np.arange(0, DH, 2), np.arange(1, DH, 2)])
    Wqh = Wqkv[0:D].reshape(H, DH, D)
    Wkh = Wqkv[D : 2 * D].reshape(H, DH, D)
    Wvh = Wqkv[2 * D : 3 * D].reshape(H, DH, D)
    scale = DH**-0.5

    half = DH // 2
    freq = THETA ** (-2.0 * np.arange(half, dtype=np.float64) / DH)
    ang = np.arange(S, dtype=np.float64)[:, None] * freq  # [S, 64]
    cosv = np.cos(ang).T.astype(np.float32)  # [64, S]
    sinv = np.sin(ang).T.astype(np.float32)
    cs = np.empty([P, 2, S], np.float32)
    cs[0:64, 0] = cosv
    cs[64:128, 0] = cosv
    cs[0:64, 1] = sinv
    cs[64:128, 1] = sinv
    cs = cs.astype(BF)

    ones = np.ones([P, P], BF)
    pp = np.arange(P)[:, None]
    cc = np.arange(P)[None, :]
    tri = (pp <= cc).astype(BF)  # 1 on/above diagonal (sq >= sk allowed)

    in_maps = []
    for c in range(8):
        b, g = c // 2, c % 2
        hs = slice(g * HPC, (g + 1) * HPC)
        wq = (Wqh[hs][:, perm, :] * scale).reshape(DC, D)
        wk = Wkh[hs][:, perm, :].reshape(DC, D)
        wv = Wvh[hs].reshape(DC, D)
        w1T = np.concatenate([wq, wk, wv], 0).T.astype(BF)  # [D, 3DC]
        # w1m[p, mt, kt, m] = w1T[kt*128+p, mt*128+m] (contiguous tiles)
        w1m = np.ascontiguousarray(
            w1T.reshape(16, P, 24, P).transpose(1, 2, 0, 3)
        )
        xT = np.ascontiguousarray(x[b].T.astype(BF))  # [D, S]
        woT = Wout[:, g * DC : (g + 1) * DC].T.astype(BF)  # [DC, D]
        wom = np.ascontiguousarray(woT.reshape(8, P, D).transpose(1, 0, 2))

        sl = int(seq_lens[b])
        bias = np.zeros([P, 8], np.float32)
        for t in range(4, 8):
            bias[:, t] = np.where(t * P + np.arange(P) >= sl, NEG, 0.0)
        in_maps.append(
            dict(
                xT=xT,
                w1m=w1m,
                wom=wom,
                cs=cs,
                tri=tri,
                bias=bias,
                ones=ones,
            )
        )
    return in_maps


def kernel(in_features, past_k, past_v, attention_mask, W_qkv, W_out):
    seq_max = int(np.asarray(attention_mask).astype(np.int64).sum(-1).max())
    nc = get_nc(KTOK if seq_max <= KTOK else S)
    in_maps = make_in_maps(in_features, attention_mask, W_qkv, W_out)
    res = run_bass_kernel_spmd(nc, in_maps, core_ids=list(range(8)))
    out = np.empty((B, S, D), np.float32)
    for b in range(B):
        out[b] = (
            res.results[2 * b]["outT"].astype(np.float32)
            + res.results[2 * b + 1]["outT"].astype(np.float32)
        ).T
    return out


# revision 32
# speedup vs baseline: 1.1600x; 1.1600x over previous
"""Trainium2 Bass kernel for fused attention prefill (nn_Attn_50740743635107).

Reference computation (fp32):
  qkv = x @ W_qkv.T ; split q,k,v ; interleaved RoPE on q,k ;
  scores = q k^T / sqrt(dh) with causal+valid_k mask ; softmax ;
  ctx = attn @ v ; out = ctx @ W_out.T

Shapes: B=4, S=1024, D=2048, H=16, DH=128.  seq_lens <= 928 for 3 of the
4 batches, 923 for batch 2 (deterministic inputs), but the kernel only
assumes seq_len >= 512 (mask data handles the rest).

Sharding: 8 cores = 4 batches x 2 head-groups (8 heads each).
Each core computes a partial out^T [D, S] for its (batch, head-group);
the host sums the two head-group partials per batch and transposes.

v2 design notes (vs the f32r v1):
- Everything bf16 on the PE/DVE paths: weights, x, q/k/v, exp(scores),
  ctx, W_out, output partials.  PSUM stays f32.  Halves LDWEIGHTS time,
  DVE time and DMA bytes; matmul row rate is 1 cycle/row either way.
- Host passes x^T and W^T layouts so every matmul contraction dim lands
  on SBUF partitions; q/k rows of W_qkv host-permuted so RoPE interleave
  becomes contiguous halves; 1/sqrt(dh) folded into W_q.
- k-projection only computes tokens < 928 (tail memset to 0; bias masks
  it), q full.
- exp: ONE activation per score tile (bias = valid_k column mask);
  causal masking by multiplying the diagonal 128x128 block of exp by a
  0/1 lower-triangular bf16 tile on DVE (scores are small, exp can't
  overflow, so masking after exp is exact).
- softmax denominator via ones-stationary matmul (broadcast across
  partitions); 1/den via DVE reciprocal_approx_fast (5x faster than the
  exact one, ~18 bits).
- PE warm-up: a run of dummy matmuls on the first-arrived const tile
  ramps the tensor-engine p-state while input DMAs land.
- Attention runs sh-major; the sq<512 half of the output projection is
  interleaved into the sq>=512 attention stream so the PE never waits
  on the ACT exp chain.
"""

import numpy as np
import ml_dtypes

import concourse.bass as bass
from concourse import bacc
import concourse.mybir as mybir
import concourse.tile as tile
from concourse.bass_utils import run_bass_kernel_spmd

B, S, D, H = 4, 1024, 2048, 16
DH = 128           # head dim
HPC = 8            # heads per core
DC = HPC * DH      # 1024: d-range per core
P = 128
KTOK = 928         # k tokens computed (>= max seq_len, multiple of 16)
THETA = 10000.0
NEG = -60.0        # additive mask for invalid-k positions
F32 = mybir.dt.float32
BF16 = mybir.dt.bfloat16
MULT = mybir.AluOpType.mult
ADD = mybir.AluOpType.add
SUB = mybir.AluOpType.subtract
EXP = mybir.ActivationFunctionType.Exp
COPY = mybir.ActivationFunctionType.Copy

# score tiles per head: (sk_tile, sq_half) pairs that are (partially) allowed.
# sq_half h covers sq in [512h, 512h+512); sk tile t covers sk in [128t, ...).
# diagonal (need causal mask): sh=0: t=0..3 ; sh=1: t=4..7.
# full (no mask): sh=1: t=0..3.
DIAG = {(t, 0) for t in range(4)} | {(t, 1) for t in range(4, 8)}
ALLOWED = {0: [0, 1, 2, 3], 1: [0, 1, 2, 3, 4, 5, 6, 7]}  # sq_half -> sk tiles


def build_nc(ktok=KTOK):
    nc = bacc.Bacc()
    xT_d = nc.dram_tensor("xT", [D, S], BF16, kind="ExternalInput")
    # w1m[p, mt, kt, m] = W^T[kt*128+p, mt*128+m]: every [P,16,128] weight
    # tile is contiguous per partition (fast DMA, one descriptor/partition)
    w1m_d = nc.dram_tensor("w1m", [P, 24, 16, P], BF16, kind="ExternalInput")
    wom_d = nc.dram_tensor("wom", [P, 8, D], BF16, kind="ExternalInput")
    cs_d = nc.dram_tensor("cs", [P, 2, S], BF16, kind="ExternalInput")
    tri_d = nc.dram_tensor("tri", [P, P], BF16, kind="ExternalInput")
    bias_d = nc.dram_tensor("bias", [P, 8], F32, kind="ExternalInput")
    ones_d = nc.dram_tensor("ones", [P, P], BF16, kind="ExternalInput")
    outT_d = nc.dram_tensor("outT", [D, S], BF16, kind="ExternalOutput")

    with tile.TileContext(nc) as tc:
        with (
            tc.tile_pool(name="qkt", bufs=1) as qktp,      # [128,16,1024] bf16 32K/p
            tc.tile_pool(name="vsb", bufs=1) as vsbp,      # [128,8,1024] bf16 16K/p
            tc.tile_pool(name="cstb", bufs=1) as cstbp,    # tri/bias/ones consts
        ):
            qkT = qktp.tile([P, 16, S], BF16, tag="qkt")
            vsb = vsbp.tile([P, 8, DC], BF16, tag="vsb")
            # consts on the gpsimd queue (cheap issue) so the scalar queue
            # starts streaming wqk0 immediately
            tri_t = cstbp.tile([P, P], BF16, tag="tri")
            nc.gpsimd.dma_start(tri_t[:], tri_d[:])
            bias_t = cstbp.tile([P, 8], F32, tag="bias")
            nc.gpsimd.dma_start(bias_t[:], bias_d[:])
            ones_t = cstbp.tile([P, P], BF16, tag="ones")
            nc.gpsimd.dma_start(ones_t[:], ones_d[:])

            # ---- PE p-state warm-up: dummy matmuls on a memset tile (no DMA
            # dependency) while the input DMAs land. Output is never read.
            with (
                tc.tile_pool(name="wsb", bufs=1) as wsbp,
                tc.tile_pool(name="warm", bufs=2, space=bass.MemorySpace.PSUM) as wps,
            ):
                warm_sb = wsbp.tile([P, 512], BF16, tag="wsb")
                nc.vector.memset(warm_sb[:], 0.5)
                for w in range(40):
                    wp = wps.tile([P, 512], F32, tag="warm")
                    nc.tensor.matmul(
                        wp[:], warm_sb[:, 0:P], warm_sb[:], start=True, stop=True
                    )

            # ================= phase A: QKV projection + RoPE =================
            with (
                tc.tile_pool(name="xt", bufs=1) as xtp,      # [128,16,1024] bf16 32K/p
                tc.tile_pool(name="wqk", bufs=8) as wqkp,    # [128,16,128] bf16 4K/p
                tc.tile_pool(name="wv", bufs=2) as wvp,      # [128,16,512] bf16 16K/p
                tc.tile_pool(name="cst", bufs=1) as cstp,    # cos/sin 4K/p
                tc.tile_pool(name="rope", bufs=2) as ropep,  # [128,1024] bf16 2K/p
                tc.tile_pool(name="psa", bufs=4, space=bass.MemorySpace.PSUM) as psap,
                tc.tile_pool(name="psv", bufs=2, space=bass.MemorySpace.PSUM) as psvp,
            ):
                xt = xtp.tile([P, 16, S], BF16, tag="xt")
                # zero the uncomputed k tail (tokens ktok..S of every k head)
                if ktok < S:
                    nc.vector.memset(qkT[:, 8:16, ktok:S], 0.0)
                # DMA plan (measured: the scalar-issued queue is the fastest
                # and earliest; sync's queue starts ~8us late from the NEFF
                # preamble; engine queues are in-order, so every scalar-queue
                # DMA must be issued BEFORE phase-A copies enter the ACT
                # queue).  First-needed bytes ride scalar, interleaved
                # wqk/xt; late tiles go gpsimd in-loop; sync gets only
                # late-needed data.
                cs_t = cstp.tile([P, 2, S], BF16, tag="cs")
                nc.gpsimd.dma_start(cs_t[:], cs_d[:])
                wqks = []
                SCAL_XT = [0, 1, 2, 3, 4, 6, 7, 9, 10, 12]
                GPS_XT = [5, 8, 11, 14]
                SYNC_XT = [13, 15]
                with tc.high_priority():
                    xt_order = iter(SCAL_XT)
                    for pre in range(6):
                        wqk_p = wqkp.tile(
                            [P, 16, P], BF16, tag="wqk", name=f"wqk_pre{pre}"
                        )
                        nc.scalar.dma_start(wqk_p[:], w1m_d[:, pre])
                        wqks.append(wqk_p)
                        if pre < 2:
                            kt = next(xt_order)
                            nc.scalar.dma_start(
                                xt[:, kt, :], xT_d[P * kt : P * (kt + 1), :]
                            )
                    for kt in xt_order:
                        nc.scalar.dma_start(xt[:, kt, :], xT_d[P * kt : P * (kt + 1), :])
                    for kt in GPS_XT:
                        nc.gpsimd.dma_start(xt[:, kt, :], xT_d[P * kt : P * (kt + 1), :])
                    for kt in SYNC_XT:
                        nc.sync.dma_start(xt[:, kt, :], xT_d[P * kt : P * (kt + 1), :])

                # ---- q/k projection (m-tile mt: 0..7 = q heads, 8..15 = k heads)
                # qkT[m, s] = sum_d w1T[d, m] * xT[d, s]
                for mt in range(16):
                    if mt < 6:
                        wqk = wqks[mt]
                    else:
                        wqk = wqkp.tile([P, 16, P], BF16, tag="wqk")
                        nc.gpsimd.dma_start(wqk[:], w1m_d[:, mt])
                    is_k = mt >= 8
                    n1 = (ktok - 512) if is_k else 512  # second-half cols
                    ps0 = psap.tile([P, 512], F32, tag="psa", name=f"qk{mt}_0")
                    ps1 = psap.tile([P, 512], F32, tag="psa", name=f"qk{mt}_1")
                    for kt in range(16):
                        nc.tensor.matmul(
                            ps0[:], wqk[:, kt, :], xt[:, kt, 0:512],
                            start=(kt == 0), stop=(kt == 15),
                        )
                        nc.tensor.matmul(
                            ps1[:, 0:n1], wqk[:, kt, :], xt[:, kt, 512 : 512 + n1],
                            start=(kt == 0), stop=(kt == 15),
                        )
                    nc.scalar.activation(qkT[:, mt, 0:512], ps0[:], COPY)
                    nc.scalar.activation(
                        qkT[:, mt, 512 : 512 + n1], ps1[:, 0:n1], COPY
                    )
                    # ---- RoPE in place on qkT[:, mt, :] once both halves done.
                    # rows 0..63 = even dh (xe), 64..127 = odd dh (xo):
                    #   new_e = xe*cos - xo*sin ; new_o = xe*sin + xo*cos
                    tmp = ropep.tile([P, S], BF16, tag="rope")
                    col = qkT[:, mt, :]
                    nc.vector.tensor_tensor(
                        tmp[0:64, :], col[64:128, :], cs_t[64:128, 1, :], op=MULT
                    )
                    nc.vector.tensor_tensor(
                        tmp[64:128, :], col[0:64, :], cs_t[0:64, 1, :], op=MULT
                    )
                    nc.vector.tensor_tensor(col[:], col[:], cs_t[:, 0, :], op=MULT)
                    nc.vector.tensor_tensor(
                        col[0:64, :], col[0:64, :], tmp[0:64, :], op=SUB
                    )
                    nc.vector.tensor_tensor(
                        col[64:128, :], col[64:128, :], tmp[64:128, :], op=ADD
                    )

                # ---- v projection: v[s, vd] = sum_d xT[d, s] * w1T[d, 2048+vd]
                # 512-wide moving chunks (full PSUM bank per matmul)
                for nh in range(2):
                    wv = wvp.tile([P, 4, 16, P], BF16, tag="wv")
                    nc.gpsimd.dma_start(wv[:], w1m_d[:, 16 + 4 * nh : 20 + 4 * nh])
                    for st in range(8):
                        psv = psvp.tile([P, 512], F32, tag="psv")
                        for kt in range(16):
                            nc.tensor.matmul(
                                psv[:],
                                xt[:, kt, P * st : P * (st + 1)],
                                wv[:, :, kt, :],
                                start=(kt == 0),
                                stop=(kt == 15),
                            )
                        nc.scalar.activation(
                            vsb[:, st, 512 * nh : 512 * (nh + 1)], psv[:], COPY
                        )

            # ============ phase B: attention + output projection ============
            with (
                tc.tile_pool(name="ctx", bufs=1) as ctxp,    # [128,8,1024] bf16 16K/p
                tc.tile_pool(name="ex", bufs=4) as exps,     # [128,512] bf16 1K/p
                tc.tile_pool(name="rc", bufs=2) as rcp,      # [128,512] f32 2K/p
                tc.tile_pool(name="wo", bufs=1) as wop,      # [128,8,2048] bf16 32K/p
                tc.tile_pool(name="ot", bufs=3) as otp,      # [128,512] bf16 1K/p
                tc.tile_pool(name="ps", bufs=3, space=bass.MemorySpace.PSUM) as psp,
                tc.tile_pool(name="psc", bufs=2, space=bass.MemorySpace.PSUM) as pscp,
                tc.tile_pool(name="psd", bufs=1, space=bass.MemorySpace.PSUM) as psdp,
                tc.tile_pool(name="pso", bufs=2, space=bass.MemorySpace.PSUM) as psop,
            ):
                ctxT = ctxp.tile([P, 8, S], BF16, tag="ctx")
                # W_out^T resident for the output projection
                wo_t = wop.tile([P, 8, D], BF16, tag="wo")
                nc.sync.dma_start(wo_t[:], wom_d[:])

                # ---- attention, software-pipelined and sh-major: scores
                # issue LOOKAHEAD items ahead of their exp/ctx/den; the sh=0
                # half of the output projection is interleaved into the sh=1
                # attention stream to keep the PE busy under the ACT chain.
                work = []  # (h, sh, t, i, ntiles)
                for sh in range(2):
                    for h in range(8):
                        tiles = ALLOWED[sh]
                        for i, t in enumerate(tiles):
                            work.append((h, sh, t, i, len(tiles)))

                LOOKAHEAD = 2
                scs = {}
                groups = {}  # (h, sh) -> (ctx_ps, den_ps)

                def issue_score(j):
                    h, sh, t, i, _n = work[j]
                    c0 = P * t - 512 * sh if (t, sh) in DIAG else 0
                    sc = psp.tile([P, 512], F32, tag="ps")
                    nc.tensor.matmul(
                        sc[:, c0:512],
                        qkT[:, 8 + h, P * t : P * (t + 1)],
                        qkT[:, h, 512 * sh + c0 : 512 * (sh + 1)],
                        start=True,
                        stop=True,
                    )
                    scs[j] = sc

                def out_proj(me, sh):
                    # outT[e, sq] = sum_d woT[d, e] * ctxT[d, sq]
                    ps = psop.tile([P, 512], F32, tag="pso")
                    for kd in range(8):
                        nc.tensor.matmul(
                            ps[:],
                            wo_t[:, kd, P * me : P * (me + 1)],
                            ctxT[:, kd, 512 * sh : 512 * (sh + 1)],
                            start=(kd == 0),
                            stop=(kd == 7),
                        )
                    ot = otp.tile([P, 512], BF16, tag="ot")
                    if me % 2 == 0:
                        nc.scalar.activation(ot[:], ps[:], COPY)
                    else:
                        nc.vector.tensor_copy(ot[:], ps[:])
                    # split the store across two queues to shorten the tail
                    d0 = outT_d[P * me : P * (me + 1), 512 * sh : 512 * sh + 256]
                    d1 = outT_d[P * me : P * (me + 1), 512 * sh + 256 : 512 * (sh + 1)]
                    if me % 2 == 0:
                        nc.sync.dma_start(d0, ot[:, 0:256])
                        nc.gpsimd.dma_start(d1, ot[:, 256:512])
                    else:
                        nc.gpsimd.dma_start(d0, ot[:, 0:256])
                        nc.sync.dma_start(d1, ot[:, 256:512])

                me_sh0 = 0  # next out-proj column tile for the sh=0 half
                for j in range(min(LOOKAHEAD, len(work))):
                    issue_score(j)
                for j, (h, sh, t, i, ntiles) in enumerate(work):
                    if j + LOOKAHEAD < len(work):
                        issue_score(j + LOOKAHEAD)
                    sc = scs.pop(j)
                    diag = (t, sh) in DIAG
                    c0 = P * t - 512 * sh if diag else 0
                    ex = exps.tile([P, 512], BF16, tag="ex")
                    bias = bias_t[:, t : t + 1] if (sh == 1 and t >= 4) else 0.0
                    nc.scalar.activation(ex[:, c0:512], sc[:, c0:512], EXP, bias=bias)
                    if diag:
                        # causal mask: zero the upper triangle of the
                        # diagonal 128x128 block (exact: 0 * finite = 0)
                        nc.vector.tensor_tensor(
                            ex[:, c0 : c0 + P], ex[:, c0 : c0 + P], tri_t[:], op=MULT
                        )
                    if i == 0:
                        ctx_ps = pscp.tile([P, 512], F32, tag="psc", name=f"ctxps_{h}_{sh}")
                        den_ps = psdp.tile([P, 512], F32, tag="psd", name=f"denps_{h}_{sh}")
                        groups[(h, sh)] = (ctx_ps, den_ps)
                    ctx_ps, den_ps = groups[(h, sh)]
                    first, last = (i == 0), (i == ntiles - 1)
                    nc.tensor.matmul(
                        ctx_ps[:, c0:512],
                        vsb[:, t, DH * h : DH * (h + 1)],
                        ex[:, c0:512],
                        start=first,
                        stop=last,
                    )
                    nc.tensor.matmul(
                        den_ps[:, c0:512], ones_t[:], ex[:, c0:512], start=first, stop=last
                    )
                    if last:
                        rc = rcp.tile([P, 512], F32, tag="rc")
                        nc.vector.reciprocal_approx_fast(rc[:], den_ps[:])
                        nc.vector.tensor_tensor(
                            ctxT[:, h, 512 * sh : 512 * (sh + 1)],
                            ctx_ps[:],
                            rc[:],
                            op=MULT,
                        )
                    # interleave the sh=0 output projection into the sh=1
                    # attention stream (2 column tiles per work item)
                    if sh == 1 and i >= ntiles - 2 and me_sh0 < 16:
                        out_proj(me_sh0, 0)
                        me_sh0 += 1

                while me_sh0 < 16:
                    out_proj(me_sh0, 0)
                    me_sh0 += 1
                for me in range(16):
                    out_proj(me, 1)
    nc.finalize()
    return nc


_NC_CACHE = {}


def get_nc(ktok=KTOK):
    if ktok not in _NC_CACHE:
        _NC_CACHE[ktok] = build_nc(ktok)
    return _NC_CACHE[ktok]


def make_in_maps(in_features, attention_mask, W_qkv, W_out):
    BF = ml_dtypes.bfloat16
    x = np.asarray(in_features, np.float32)
    am = np.asarray(attention_mask)
    Wqkv = np.asarray(W_qkv, np.float32)
    Wout = np.asarray(W_out, np.float32)
    seq_lens = am.astype(np.int64).sum(-1)

    perm = np.concatenate([np.arange(0, DH, 2), np.arange(1, DH, 2)])
    Wqh = Wqkv[0:D].reshape(H, DH, D)
    Wkh = Wqkv[D : 2 * D].reshape(H, DH, D)
    Wvh = Wqkv[2 * D : 3 * D].reshape(H, DH, D)
    scale = DH**-0.5

    half = DH // 2
    freq = THETA ** (-2.0 * np.arange(half, dtype=np.float64) / DH)
    ang = np.arange(S, dtype=np.float64)[:, None] * freq  # [S, 64]
    cosv = np.cos(ang).T.astype(np.float32)  # [64, S]
    sinv = np.sin(ang).T.astype(np.float32)
    cs = np.empty([P, 2, S], np.float32)
    cs[0:64, 0] = cosv
    cs[64:128, 0] = cosv
    cs[0:64, 1] = sinv
    cs[64:128, 1] = sinv
    cs = cs.astype(BF)

    ones = np.ones([P, P], BF)
    pp = np.arange(P)[:, None]
    cc = np.arange(P)[None, :]
    tri = (pp <= cc).astype(BF)  # 1 on/above diagonal (sq >= sk allowed)

    in_maps = []
    for c in range(8):
        b, g = c // 2, c % 2
        hs = slice(g * HPC, (g + 1) * HPC)
        wq = (Wqh[hs][:, perm, :] * scale).reshape(DC, D)
        wk = Wkh[hs][:, perm, :].reshape(DC, D)
        wv = Wvh[hs].reshape(DC, D)
        w1T = np.concatenate([wq, wk, wv], 0).T.astype(BF)  # [D, 3DC]
        # w1m[p, mt, kt, m] = w1T[kt*128+p, mt*128+m] (contiguous tiles)
        w1m = np.ascontiguousarray(
            w1T.reshape(16, P, 24, P).transpose(1, 2, 0, 3)
        )
        xT = np.ascontiguousarray(x[b].T.astype(BF))  # [D, S]
        woT = Wout[:, g * DC : (g + 1) * DC].T.astype(BF)  # [DC, D]
        wom = np.ascontiguousarray(woT.reshape(8, P, D).transpose(1, 0, 2))

        sl = int(seq_lens[b])
        bias = np.zeros([P, 8], np.float32)
        for t in range(4, 8):
            bias[:, t] = np.where(t * P + np.arange(P) >= sl, NEG, 0.0)
        in_maps.append(
            dict(
                xT=xT,
                w1m=w1m,
                wom=wom,
                cs=cs,
                tri=tri,
                bias=bias,
                ones=ones,
            )
        )
    return in_maps


def kernel(in_features, past_k, past_v, attention_mask, W_qkv, W_out):
    seq_max = int(np.asarray(attention_mask).astype(np.int64).sum(-1).max())
    nc = get_nc(KTOK if seq_max <= KTOK else S)
    in_maps = make_in_maps(in_features, attention_mask, W_qkv, W_out)
    res = run_bass_kernel_spmd(nc, in_maps, core_ids=list(range(8)))
    out = np.empty((B, S, D), np.float32)
    for b in range(B):
        out[b] = (
            res.results[2 * b]["outT"].astype(np.float32)
            + res.results[2 * b + 1]["outT"].astype(np.float32)
        ).T
    return out


# revision 38
# speedup vs baseline: 1.1770x; 1.0147x over previous
"""Trainium2 Bass kernel for fused attention prefill (nn_Attn_50740743635107).

Reference computation (fp32):
  qkv = x @ W_qkv.T ; split q,k,v ; interleaved RoPE on q,k ;
  scores = q k^T / sqrt(dh) with causal+valid_k mask ; softmax ;
  ctx = attn @ v ; out = ctx @ W_out.T

Shapes: B=4, S=1024, D=2048, H=16, DH=128.  seq_lens <= 928 for 3 of the
4 batches, 923 for batch 2 (deterministic inputs), but the kernel only
assumes seq_len >= 512 (mask data handles the rest).

Sharding: 8 cores = 4 batches x 2 head-groups (8 heads each).
Each core computes a partial out^T [D, S] for its (batch, head-group);
the host sums the two head-group partials per batch and transposes.

v2 design notes (vs the f32r v1):
- Everything bf16 on the PE/DVE paths: weights, x, q/k/v, exp(scores),
  ctx, W_out, output partials.  PSUM stays f32.  Halves LDWEIGHTS time,
  DVE time and DMA bytes; matmul row rate is 1 cycle/row either way.
- Host passes x^T and W^T layouts so every matmul contraction dim lands
  on SBUF partitions; q/k rows of W_qkv host-permuted so RoPE interleave
  becomes contiguous halves; 1/sqrt(dh) folded into W_q.
- k-projection only computes tokens < 928 (tail memset to 0; bias masks
  it), q full.
- exp: ONE activation per score tile (bias = valid_k column mask);
  causal masking by multiplying the diagonal 128x128 block of exp by a
  0/1 lower-triangular bf16 tile on DVE (scores are small, exp can't
  overflow, so masking after exp is exact).
- softmax denominator via ones-stationary matmul (broadcast across
  partitions); 1/den via DVE reciprocal_approx_fast (5x faster than the
  exact one, ~18 bits).
- PE warm-up: a run of dummy matmuls on the first-arrived const tile
  ramps the tensor-engine p-state while input DMAs land.
- Attention runs sh-major; the sq<512 half of the output projection is
  interleaved into the sq>=512 attention stream so the PE never waits
  on the ACT exp chain.
"""

import numpy as np
import ml_dtypes

import concourse.bass as bass
from concourse import bacc
import concourse.mybir as mybir
import concourse.tile as tile
from concourse.bass_utils import run_bass_kernel_spmd

B, S, D, H = 4, 1024, 2048, 16
DH = 128           # head dim
HPC = 8            # heads per core
DC = HPC * DH      # 1024: d-range per core
P = 128
KTOK = 928         # k tokens computed (>= max seq_len, multiple of 16)
THETA = 10000.0
NEG = -60.0        # additive mask for invalid-k positions
F32 = mybir.dt.float32
BF16 = mybir.dt.bfloat16
MULT = mybir.AluOpType.mult
ADD = mybir.AluOpType.add
SUB = mybir.AluOpType.subtract
EXP = mybir.ActivationFunctionType.Exp
COPY = mybir.ActivationFunctionType.Copy

# score tiles per head: (sk_tile, sq_half) pairs that are (partially) allowed.
# sq_half h covers sq in [512h, 512h+512); sk tile t covers sk in [128t, ...).
# diagonal (need causal mask): sh=0: t=0..3 ; sh=1: t=4..7.
# full (no mask): sh=1: t=0..3.
DIAG = {(t, 0) for t in range(4)} | {(t, 1) for t in range(4, 8)}
ALLOWED = {0: [0, 1, 2, 3], 1: [0, 1, 2, 3, 4, 5, 6, 7]}  # sq_half -> sk tiles


def build_nc(ktok=KTOK):
    nc = bacc.Bacc()
    xT_d = nc.dram_tensor("xT", [D, S], BF16, kind="ExternalInput")
    # w1m[p, mt, kt, m] = W^T[kt*128+p, mt*128+m]: every [P,16,128] weight
    # tile is contiguous per partition (fast DMA, one descriptor/partition)
    w1m_d = nc.dram_tensor("w1m", [P, 24, 16, P], BF16, kind="ExternalInput")
    wom_d = nc.dram_tensor("wom", [P, 8, D], BF16, kind="ExternalInput")
    cs_d = nc.dram_tensor("cs", [P, 2, S], BF16, kind="ExternalInput")
    tri_d = nc.dram_tensor("tri", [P, P], BF16, kind="ExternalInput")
    bias_d = nc.dram_tensor("bias", [P, 8], F32, kind="ExternalInput")
    ones_d = nc.dram_tensor("ones", [P, P], BF16, kind="ExternalInput")
    outT_d = nc.dram_tensor("outT", [D, S], BF16, kind="ExternalOutput")

    with tile.TileContext(nc) as tc:
        with (
            tc.tile_pool(name="qkt", bufs=1) as qktp,      # [128,16,1024] bf16 32K/p
            tc.tile_pool(name="vsb", bufs=1) as vsbp,      # [128,8,1024] bf16 16K/p
            tc.tile_pool(name="cstb", bufs=1) as cstbp,    # tri/bias/ones consts
        ):
            qkT = qktp.tile([P, 16, S], BF16, tag="qkt")
            vsb = vsbp.tile([P, 8, DC], BF16, tag="vsb")
            # consts on the gpsimd queue (cheap issue) so the scalar queue
            # starts streaming wqk0 immediately
            tri_t = cstbp.tile([P, P], BF16, tag="tri")
            nc.gpsimd.dma_start(tri_t[:], tri_d[:])
            bias_t = cstbp.tile([P, 8], F32, tag="bias")
            nc.gpsimd.dma_start(bias_t[:], bias_d[:])
            ones_t = cstbp.tile([P, P], BF16, tag="ones")
            nc.gpsimd.dma_start(ones_t[:], ones_d[:])

            # ---- PE p-state warm-up: dummy matmuls on a memset tile (no DMA
            # dependency) while the input DMAs land. Output is never read.
            with (
                tc.tile_pool(name="wsb", bufs=1) as wsbp,
                tc.tile_pool(name="warm", bufs=2, space=bass.MemorySpace.PSUM) as wps,
            ):
                warm_sb = wsbp.tile([P, 512], BF16, tag="wsb")
                nc.vector.memset(warm_sb[:], 0.5)
                for w in range(24):
                    wp = wps.tile([P, 512], F32, tag="warm")
                    nc.tensor.matmul(
                        wp[:], warm_sb[:, 0:P], warm_sb[:], start=True, stop=True
                    )

            # ================= phase A: QKV projection + RoPE =================
            with (
                tc.tile_pool(name="xt", bufs=16) as xtp,     # 16x[128,1024] bf16 32K/p
                tc.tile_pool(name="wqk", bufs=8) as wqkp,    # [128,16,128] bf16 4K/p
                tc.tile_pool(name="wv", bufs=2) as wvp,      # [128,16,512] bf16 16K/p
                tc.tile_pool(name="cst", bufs=1) as cstp,    # cos/sin 4K/p
                tc.tile_pool(name="rope", bufs=2) as ropep,  # [128,1024] bf16 2K/p
                tc.tile_pool(name="psa", bufs=4, space=bass.MemorySpace.PSUM) as psap,
                tc.tile_pool(name="psv", bufs=2, space=bass.MemorySpace.PSUM) as psvp,
            ):
                # x as 16 SEPARATE tiles: tile-granular semaphores mean a
                # single [P,16,S] tile would gate the first matmul on ALL 16
                # chunk DMAs; separate tiles give per-chunk dependencies.
                xts = [
                    xtp.tile([P, S], BF16, tag="xt", name=f"xt{kt}")
                    for kt in range(16)
                ]
                # zero the uncomputed k tail (tokens ktok..S of every k head)
                if ktok < S:
                    nc.vector.memset(qkT[:, 8:16, ktok:S], 0.0)
                # DMA plan (measured: the scalar-issued queue is the fastest
                # and earliest; sync's queue starts ~8us late from the NEFF
                # preamble; engine queues are in-order, so every scalar-queue
                # DMA must be issued BEFORE phase-A copies enter the ACT
                # queue).  First-needed bytes ride scalar in consumption
                # order; late tiles go gpsimd in-loop; sync gets only
                # late-needed data.
                cs_t = cstp.tile([P, 2, S], BF16, tag="cs")
                nc.gpsimd.dma_start(cs_t[:], cs_d[:])
                wqks = []
                with tc.high_priority():
                    for pre in range(6):
                        wqk_p = wqkp.tile(
                            [P, 16, P], BF16, tag="wqk", name=f"wqk_pre{pre}"
                        )
                        nc.scalar.dma_start(wqk_p[:], w1m_d[:, pre])
                        wqks.append(wqk_p)
                        kt = pre
                        nc.scalar.dma_start(xts[kt][:], xT_d[P * kt : P * (kt + 1), :])
                    for kt in range(6, 12):
                        nc.scalar.dma_start(xts[kt][:], xT_d[P * kt : P * (kt + 1), :])
                    for kt in (12, 13, 15):
                        nc.gpsimd.dma_start(xts[kt][:], xT_d[P * kt : P * (kt + 1), :])
                    nc.sync.dma_start(xts[14][:], xT_d[14 * P : 15 * P, :])

                # ---- q/k projection (m-tile mt: 0..7 = q heads, 8..15 = k heads)
                # qkT[m, s] = sum_d w1T[d, m] * xT[d, s]
                for mt in range(16):
                    if mt < 6:
                        wqk = wqks[mt]
                    else:
                        wqk = wqkp.tile([P, 16, P], BF16, tag="wqk")
                        nc.gpsimd.dma_start(wqk[:], w1m_d[:, mt])
                    is_k = mt >= 8
                    n1 = (ktok - 512) if is_k else 512  # second-half cols
                    ps0 = psap.tile([P, 512], F32, tag="psa", name=f"qk{mt}_0")
                    ps1 = psap.tile([P, 512], F32, tag="psa", name=f"qk{mt}_1")
                    for kt in range(16):
                        nc.tensor.matmul(
                            ps0[:], wqk[:, kt, :], xts[kt][:, 0:512],
                            start=(kt == 0), stop=(kt == 15),
                        )
                        nc.tensor.matmul(
                            ps1[:, 0:n1], wqk[:, kt, :], xts[kt][:, 512 : 512 + n1],
                            start=(kt == 0), stop=(kt == 15),
                        )
                    nc.scalar.activation(qkT[:, mt, 0:512], ps0[:], COPY)
                    nc.scalar.activation(
                        qkT[:, mt, 512 : 512 + n1], ps1[:, 0:n1], COPY
                    )
                    # ---- RoPE in place on qkT[:, mt, :] once both halves done.
                    # rows 0..63 = even dh (xe), 64..127 = odd dh (xo):
                    #   new_e = xe*cos - xo*sin ; new_o = xe*sin + xo*cos
                    tmp = ropep.tile([P, S], BF16, tag="rope")
                    col = qkT[:, mt, :]
                    nc.vector.tensor_tensor(
                        tmp[0:64, :], col[64:128, :], cs_t[64:128, 1, :], op=MULT
                    )
                    nc.vector.tensor_tensor(
                        tmp[64:128, :], col[0:64, :], cs_t[0:64, 1, :], op=MULT
                    )
                    nc.vector.tensor_tensor(col[:], col[:], cs_t[:, 0, :], op=MULT)
                    nc.vector.tensor_tensor(
                        col[0:64, :], col[0:64, :], tmp[0:64, :], op=SUB
                    )
                    nc.vector.tensor_tensor(
                        col[64:128, :], col[64:128, :], tmp[64:128, :], op=ADD
                    )

                # ---- v projection: v[s, vd] = sum_d xT[d, s] * w1T[d, 2048+vd]
                # 512-wide moving chunks (full PSUM bank per matmul)
                for nh in range(2):
                    wv = wvp.tile([P, 4, 16, P], BF16, tag="wv")
                    nc.gpsimd.dma_start(wv[:], w1m_d[:, 16 + 4 * nh : 20 + 4 * nh])
                    for st in range(8):
                        psv = psvp.tile([P, 512], F32, tag="psv")
                        for kt in range(16):
                            nc.tensor.matmul(
                                psv[:],
                                xts[kt][:, P * st : P * (st + 1)],
                                wv[:, :, kt, :],
                                start=(kt == 0),
                                stop=(kt == 15),
                            )
                        nc.scalar.activation(
                            vsb[:, st, 512 * nh : 512 * (nh + 1)], psv[:], COPY
                        )

            # ============ phase B: attention + output projection ============
            with (
                tc.tile_pool(name="ctx", bufs=1) as ctxp,    # [128,8,1024] bf16 16K/p
                tc.tile_pool(name="ex", bufs=4) as exps,     # [128,512] bf16 1K/p
                tc.tile_pool(name="rc", bufs=2) as rcp,      # [128,512] f32 2K/p
                tc.tile_pool(name="wo", bufs=1) as wop,      # [128,8,2048] bf16 32K/p
                tc.tile_pool(name="ot", bufs=3) as otp,      # [128,512] bf16 1K/p
                tc.tile_pool(name="ps", bufs=3, space=bass.MemorySpace.PSUM) as psp,
                tc.tile_pool(name="psc", bufs=2, space=bass.MemorySpace.PSUM) as pscp,
                tc.tile_pool(name="psd", bufs=1, space=bass.MemorySpace.PSUM) as psdp,
                tc.tile_pool(name="pso", bufs=2, space=bass.MemorySpace.PSUM) as psop,
            ):
                ctxT = ctxp.tile([P, 8, S], BF16, tag="ctx")
                # W_out^T resident for the output projection
                wo_t = wop.tile([P, 8, D], BF16, tag="wo")
                nc.sync.dma_start(wo_t[:], wom_d[:])

                # ---- attention, software-pipelined and sh-major: scores
                # issue LOOKAHEAD items ahead of their exp/ctx/den; the sh=0
                # half of the output projection is interleaved into the sh=1
                # attention stream to keep the PE busy under the ACT chain.
                work = []  # (h, sh, t, i, ntiles)
                for sh in range(2):
                    for h in range(8):
                        tiles = ALLOWED[sh]
                        for i, t in enumerate(tiles):
                            work.append((h, sh, t, i, len(tiles)))

                LOOKAHEAD = 2
                scs = {}
                groups = {}  # (h, sh) -> (ctx_ps, den_ps)

                def issue_score(j):
                    h, sh, t, i, _n = work[j]
                    c0 = P * t - 512 * sh if (t, sh) in DIAG else 0
                    sc = psp.tile([P, 512], F32, tag="ps")
                    nc.tensor.matmul(
                        sc[:, c0:512],
                        qkT[:, 8 + h, P * t : P * (t + 1)],
                        qkT[:, h, 512 * sh + c0 : 512 * (sh + 1)],
                        start=True,
                        stop=True,
                    )
                    scs[j] = sc

                def out_proj(me, sh):
                    # outT[e, sq] = sum_d woT[d, e] * ctxT[d, sq]
                    ps = psop.tile([P, 512], F32, tag="pso")
                    for kd in range(8):
                        nc.tensor.matmul(
                            ps[:],
                            wo_t[:, kd, P * me : P * (me + 1)],
                            ctxT[:, kd, 512 * sh : 512 * (sh + 1)],
                            start=(kd == 0),
                            stop=(kd == 7),
                        )
                    ot = otp.tile([P, 512], BF16, tag="ot")
                    if me % 2 == 0:
                        nc.scalar.activation(ot[:], ps[:], COPY)
                    else:
                        nc.vector.tensor_copy(ot[:], ps[:])
                    # split the store across two queues to shorten the tail
                    d0 = outT_d[P * me : P * (me + 1), 512 * sh : 512 * sh + 256]
                    d1 = outT_d[P * me : P * (me + 1), 512 * sh + 256 : 512 * (sh + 1)]
                    if me % 2 == 0:
                        nc.sync.dma_start(d0, ot[:, 0:256])
                        nc.gpsimd.dma_start(d1, ot[:, 256:512])
                    else:
                        nc.gpsimd.dma_start(d0, ot[:, 0:256])
                        nc.sync.dma_start(d1, ot[:, 256:512])

                me_sh0 = 0  # next out-proj column tile for the sh=0 half
                for j in range(min(LOOKAHEAD, len(work))):
                    issue_score(j)
                for j, (h, sh, t, i, ntiles) in enumerate(work):
                    if j + LOOKAHEAD < len(work):
                        issue_score(j + LOOKAHEAD)
                    sc = scs.pop(j)
                    diag = (t, sh) in DIAG
                    c0 = P * t - 512 * sh if diag else 0
                    ex = exps.tile([P, 512], BF16, tag="ex")
                    bias = bias_t[:, t : t + 1] if (sh == 1 and t >= 4) else 0.0
                    nc.scalar.activation(ex[:, c0:512], sc[:, c0:512], EXP, bias=bias)
                    if diag:
                        # causal mask: zero the upper triangle of the
                        # diagonal 128x128 block (exact: 0 * finite = 0)
                        nc.vector.tensor_tensor(
                            ex[:, c0 : c0 + P], ex[:, c0 : c0 + P], tri_t[:], op=MULT
                        )
                    if i == 0:
                        ctx_ps = pscp.tile([P, 512], F32, tag="psc", name=f"ctxps_{h}_{sh}")
                        den_ps = psdp.tile([P, 512], F32, tag="psd", name=f"denps_{h}_{sh}")
                        groups[(h, sh)] = (ctx_ps, den_ps)
                    ctx_ps, den_ps = groups[(h, sh)]
                    first, last = (i == 0), (i == ntiles - 1)
                    nc.tensor.matmul(
                        ctx_ps[:, c0:512],
                        vsb[:, t, DH * h : DH * (h + 1)],
                        ex[:, c0:512],
                        start=first,
                        stop=last,
                    )
                    nc.tensor.matmul(
                        den_ps[:, c0:512], ones_t[:], ex[:, c0:512], start=first, stop=last
                    )
                    if last:
                        rc = rcp.tile([P, 512], F32, tag="rc")
                        nc.vector.reciprocal_approx_fast(rc[:], den_ps[:])
                        nc.vector.tensor_tensor(
                            ctxT[:, h, 512 * sh : 512 * (sh + 1)],
                            ctx_ps[:],
                            rc[:],
                            op=MULT,
                        )
                    # interleave the sh=0 output projection into the sh=1
                    # attention stream (2 column tiles per work item)
                    if sh == 1 and i >= ntiles - 2 and me_sh0 < 16:
                        out_proj(me_sh0, 0)
                        me_sh0 += 1

                while me_sh0 < 16:
                    out_proj(me_sh0, 0)
                    me_sh0 += 1
                for me in range(16):
                    out_proj(me, 1)
    nc.finalize()
    return nc


_NC_CACHE = {}


def get_nc(ktok=KTOK):
    if ktok not in _NC_CACHE:
        _NC_CACHE[ktok] = build_nc(ktok)
    return _NC_CACHE[ktok]


def make_in_maps(in_features, attention_mask, W_qkv, W_out):
    BF = ml_dtypes.bfloat16
    x = np.asarray(in_features, np.float32)
    am = np.asarray(attention_mask)
    Wqkv = np.asarray(W_qkv, np.float32)
    Wout = np.asarray(W_out, np.float32)
    seq_lens = am.astype(np.int64).sum(-1)

    perm = np.concatenate([np.arange(0, DH, 2), np.arange(1, DH, 2)])
    Wqh = Wqkv[0:D].reshape(H, DH, D)
    Wkh = Wqkv[D : 2 * D].reshape(H, DH, D)
    Wvh = Wqkv[2 * D : 3 * D].reshape(H, DH, D)
    scale = DH**-0.5

    half = DH // 2
    freq = THETA ** (-2.0 * np.arange(half, dtype=np.float64) / DH)
    ang = np.arange(S, dtype=np.float64)[:, None] * freq  # [S, 64]
    cosv = np.cos(ang).T.astype(np.float32)  # [64, S]
    sinv = np.sin(ang).T.astype(np.float32)
    cs = np.empty([P, 2, S], np.float32)
    cs[0:64, 0] = cosv
    cs[64:128, 0] = cosv
    cs[0:64, 1] = sinv
    cs[64:128, 1] = sinv
    cs = cs.astype(BF)

    ones = np.ones([P, P], BF)
    pp = np.arange(P)[:, None]
    cc = np.arange(P)[None, :]
    tri = (pp <= cc).astype(BF)  # 1 on/above diagonal (sq >= sk allowed)

    in_maps = []
    for c in range(8):
        b, g = c // 2, c % 2
        hs = slice(g * HPC, (g + 1) * HPC)
        wq = (Wqh[hs][:, perm, :] * scale).reshape(DC, D)
        wk = Wkh[hs][:, perm, :].reshape(DC, D)
        wv = Wvh[hs].reshape(DC, D)
        w1T = np.concatenate([wq, wk, wv], 0).T.astype(BF)  # [D, 3DC]
        # w1m[p, mt, kt, m] = w1T[kt*128+p, mt*128+m] (contiguous tiles)
        w1m = np.ascontiguousarray(
            w1T.reshape(16, P, 24, P).transpose(1, 2, 0, 3)
        )
        xT = np.ascontiguousarray(x[b].T.astype(BF))  # [D, S]
        woT = Wout[:, g * DC : (g + 1) * DC].T.astype(BF)  # [DC, D]
        wom = np.ascontiguousarray(woT.reshape(8, P, D).transpose(1, 0, 2))

        sl = int(seq_lens[b])
        bias = np.zeros([P, 8], np.float32)
        for t in range(4, 8):
            bias[:, t] = np.where(t * P + np.arange(P) >= sl, NEG, 0.0)
        in_maps.append(
            dict(
                xT=xT,
                w1m=w1m,
                wom=wom,
                cs=cs,
                tri=tri,
                bias=bias,
                ones=ones,
            )
        )
    return in_maps


def kernel(in_features, past_k, past_v, attention_mask, W_qkv, W_out):
    seq_max = int(np.asarray(attention_mask).astype(np.int64).sum(-1).max())
    nc = get_nc(KTOK if seq_max <= KTOK else S)
    in_maps = make_in_maps(in_features, attention_mask, W_qkv, W_out)
    res = run_bass_kernel_spmd(nc, in_maps, core_ids=list(range(8)))
    out = np.empty((B, S, D), np.float32)
    for b in range(B):
        out[b] = (
            res.results[2 * b]["outT"].astype(np.float32)
            + res.results[2 * b + 1]["outT"].astype(np.float32)
        ).T
    return out
